# revision 55
# baseline (speedup 1.0000x reference)
"""MHA kernel for Trainium2, 8 NeuronCores — dense ping-pong pipeline v4.

Problem: B=4, T=2048, D=1024, H=16, HD=64 fp32 multi-head attention
  qkv = x @ w_qkv ; attention per head ; out = y @ w_o

Sharding: core c handles batch b = c//2 and head-group g = c%2 (8 of the 16
heads). Each core computes its 8 heads' attention output projected through
the matching w_o row-slice, producing a partial [T, D] f16 output; the host
sums the two partials per batch (row-parallel output projection).

v4 structure: 256 micro-slots gs = (window w = gs//32, s-tile i = (gs//2)%16,
t-half u = gs%2). Two PSUM score tiles [128, 1024] ping-pong by gs parity,
so scores(gs+1) overlap exp(gs) with NO write-after-read stall — the ACT
engine streams one [128, 1024] exp per slot back-to-back (1146ns), and the
PE stays ~100% busy (which also keeps the HAM clock-gate at 2.4 GHz: the
v3 variant measured 42% of the kernel at K=4/8 half-clock because per-slot
idle gaps re-throttled the array).

Per slot gs, emission (= program) order:
  ACT: exp(gs)            reads sc[gs%2] (A-head cols 0:512 | B cols 512:)
  DVE: acc_u(gs) += e(gs) f16 exp-sum per t-half
  PE : yu pair (gs-LAG)   col-tiled A||B concurrent into yu [128, 1024]
  PE : fills              QKV / O projection chains, deadline-ordered
  PE : scores(gs+1)       one row-tiled A||B concurrent pair into sc[1-gs%2]
At i==15 per u: denominator ones-matmuls ([1,512] chunks via aux psum) +
reciprocal into rec[1,2048]; at u==1 one gpsimd partition_broadcast to
bc[128,2048]; the [64,1024] normalize multiply per head lands in yt when
that head's yu finishes (LAG slots later).
"""
import sys

if "/opt/trn_rl_repo" not in sys.path:
    sys.path.insert(0, "/opt/trn_rl_repo")

import heapq

import numpy as np

import concourse.bass as bass
import concourse.mybir as mybir
import concourse.tile as tile
from concourse import bacc
from concourse.bass_isa import ReduceOp
from concourse.bass_utils import run_bass_kernel_spmd

T = 2048
D = 1024
NH = 8          # heads per core
HD = 64
KC = D // 128   # 8 contraction chunks
TT = T // 128   # 16 s tiles
NP = NH // 2    # 4 head pairs
NW = 2 * NP     # 8 windows: w = 2*p + tb
NG = NW * TT * 2  # 256 micro-slots
LAG = 6         # yu lags exp by LAG micro-slots
F32 = mybir.dt.float32
F16 = mybir.dt.float16

_CACHE = {}
_DEBUG = False


def build_nc():
    nc = bacc.Bacc(
        "TRN2",
        target_bir_lowering=False,
        debug=False,
        enable_asserts=False,
        num_devices=8,
    )
    x_d = nc.dram_tensor("x", [T, D], F16, kind="ExternalInput")
    wq_d = nc.dram_tensor("wq", [D, 512], F16, kind="ExternalInput")
    wk_d = nc.dram_tensor("wk", [D, 512], F16, kind="ExternalInput")
    wv_d = nc.dram_tensor("wv", [D, 512], F16, kind="ExternalInput")
    wo_d = nc.dram_tensor("wo", [512, D], F16, kind="ExternalInput")
    out_d = nc.dram_tensor("out", [T, D], F16, kind="ExternalOutput")
    if _DEBUG:
        qkt_d = nc.dram_tensor("qkt_dump", [128, 8, T], F16,
                               kind="ExternalOutput")
        v_d = nc.dram_tensor("v_dump", [128, TT, 512], F16,
                             kind="ExternalOutput")
        yt_d = nc.dram_tensor("yt_dump", [128, NP, T], F16,
                              kind="ExternalOutput")

    x_ap = x_d.ap()
    wq_ap = wq_d.ap().rearrange("(kc p) j -> p kc j", p=128)   # [128, 8, 512]
    wk_ap = wk_d.ap().rearrange("(kc p) j -> p kc j", p=128)
    wv_ap = wv_d.ap().rearrange("(kc p) j -> p kc j", p=128)
    wo_ap = wo_d.ap().rearrange("(c p) n -> p c n", p=128)     # [128, 4, 1024]

    def win(gs):
        """micro-slot -> (pair, tb, i, u); u-outer within a window."""
        w, r = gs // 32, gs % 32
        return w // 2, w % 2, r % 16, r // 16

    with tile.TileContext(nc) as tc:
        with (
            tc.sbuf_pool(name="sb", bufs=1) as sb,
            tc.psum_pool(name="ps", bufs=1) as ps,
        ):
            # ---- persistent sbuf (separate tiles per logical slice) ----
            xt = [sb.tile([128, T], F16, name=f"xt{kc}") for kc in range(KC)]
            qkt = [sb.tile([128, T], F16, name=f"qkt{jt}") for jt in range(8)]
            v_sb = sb.tile([128, TT, 512], F16)      # V [s-part, s-chunk, j]
            yt = sb.tile([128, NP, T], F16)          # y^T [dy, pair, t]
            wq_sb = sb.tile([128, KC, 512], F16)
            wk_sb = sb.tile([128, KC, 512], F16)
            wv_sb = sb.tile([128, KC, 512], F16)
            wo_sb = sb.tile([128, 4, D], F16)
            ones_v = sb.tile([128, 1], F16)
            nc.vector.memset(ones_v, 1.0)
            warm = sb.tile([1, 32], F16)
            nc.vector.memset(warm, 0.0)
            nc.scalar.activation(
                warm, warm, mybir.ActivationFunctionType.Exp, scale=0.125
            )

            nc.scalar.dma_start(out=wk_sb, in_=wk_ap)

            # ---------- fill chains (QKV / O projections) ----------
            fills = []     # heap of (deadline_slot, seq, key)
            fseq = [0]
            pending = {}

            def g_qk(jt, tbc):
                """qkt[jt][:, tbc*512:(tbc+1)*512] = (w chunk)^T @ xt."""
                aux = ps.tile([128, 512], F32, name="qkps", tag="aux", bufs=2)
                w_sb = wq_sb if jt < 4 else wk_sb
                j4 = jt % 4
                for kc in range(KC):
                    nc.tensor.matmul(
                        aux,
                        w_sb[:, kc, j4 * 128:(j4 + 1) * 128],
                        xt[kc][:, tbc * 512:(tbc + 1) * 512],
                        start=(kc == 0),
                        stop=(kc == KC - 1),
                        skip_group_check=True,
                    )
                    yield 230
                nc.vector.tensor_copy(
                    out=qkt[jt][:, tbc * 512:(tbc + 1) * 512], in_=aux
                )

            def g_v(i):
                aux = ps.tile([128, 512], F32, name="vps", tag="aux", bufs=2)
                for kc in range(KC):
                    nc.tensor.matmul(
                        aux,
                        xt[kc][:, i * 128:(i + 1) * 128],
                        wv_sb[:, kc, :],
                        start=(kc == 0),
                        stop=(kc == KC - 1),
                        skip_group_check=True,
                    )
                    yield 230
                nc.vector.tensor_copy(out=v_sb[:, i, :], in_=aux)

            def g_o(tt, u):
                aux = ps.tile([128, 512], F32, name="ops", tag="aux", bufs=2)
                for c4 in range(4):
                    nc.tensor.matmul(
                        aux,
                        yt[:, c4, tt * 128:(tt + 1) * 128],
                        wo_sb[:, c4, u * 512:(u + 1) * 512],
                        start=(c4 == 0),
                        stop=(c4 == 3),
                        skip_group_check=True,
                    )
                    yield 230
                o_sb = sb.tile([128, 512], F16, tag="osb", bufs=2)
                with nc.allow_low_precision(reason="f16 partial output"):
                    nc.vector.tensor_copy(out=o_sb, in_=aux)
                nc.sync.dma_start(
                    out=out_d.ap()[
                        tt * 128:(tt + 1) * 128, u * 512:(u + 1) * 512
                    ],
                    in_=o_sb,
                )

            def push_fill(key, gen, deadline=100000):
                pending[key] = gen
                heapq.heappush(fills, (deadline, fseq[0], key))
                fseq[0] += 1

            def advance_fills(budget):
                while fills and budget > 0:
                    key = fills[0][2]
                    gen = pending.get(key)
                    if gen is None:
                        heapq.heappop(fills)
                        continue
                    try:
                        budget -= next(gen)
                    except StopIteration:
                        del pending[key]
                        heapq.heappop(fills)

            def need(key):
                gen = pending.pop(key, None)
                if gen is not None:
                    for _ in gen:
                        pass

            def force_chain(gen):
                for _ in gen:
                    pass

            # ---------- attention state ----------
            # two ping-pong score tiles; A-head cols 0:512, B cols 512:1024
            scb = [ps.tile([128, 1024], F32, name=f"sc{b}", tag=f"sc{b}",
                           bufs=1) for b in range(2)]
            yu = ps.tile([128, 1024], F32, name="yu", tag="yu", bufs=1)
            exp_t = {}       # gs -> sbuf exp tile [128, 1024] (A|B)
            acc_t = [None, None]   # per-u running f16 exp-sum [128, 1024]
            bc_t = {}        # (w, u) -> 1/denominator [128, 1024] (A|B)

            def emit_sc(gs):
                """scores for micro-slot gs: one row-tiled A||B pair."""
                if gs < 0 or gs >= NG:
                    return
                p, tb, i, u = win(gs)
                need(("qk", 4 + p, i // 4))
                need(("qk", p, 2 * tb + u))
                buf = scb[gs % 2]
                tcol = tb * 1024 + u * 512
                for hb in range(2):   # A then B adjacent -> concurrent
                    pb = 64 * hb
                    nc.tensor.matmul(
                        buf[:, 512 * hb:512 * (hb + 1)],
                        qkt[4 + p][pb:pb + 64, i * 128:(i + 1) * 128],
                        qkt[p][pb:pb + 64, tcol:tcol + 512],
                        start=True,
                        stop=True,
                    )

            def emit_exp(gs):
                if gs < 0 or gs >= NG:
                    return
                e = sb.tile([128, 1024], F16, tag="exp", bufs=8)
                nc.scalar.activation(
                    e, scb[gs % 2], mybir.ActivationFunctionType.Exp,
                    scale=0.125,
                )
                exp_t[gs] = e

            def emit_acc(gs):
                if gs < 0 or gs >= NG:
                    return
                p, tb, i, u = win(gs)
                a = sb.tile([128, 1024], F16, tag=f"accu{u}", bufs=2)
                if i == 0:
                    nc.vector.tensor_copy(out=a, in_=exp_t[gs])
                else:
                    with nc.allow_low_precision(reason="f16 exp-sum"):
                        nc.vector.tensor_add(out=a, in0=acc_t[u],
                                             in1=exp_t[gs])
                acc_t[u] = a
                if i == TT - 1:
                    # denominator chunks for this u: A and B halves via
                    # ones-matmul -> reciprocal -> gpsimd broadcast.
                    w = gs // 32
                    rec = sb.tile([1, 1024], F32, tag="rec", bufs=2)
                    bc = sb.tile([128, 1024], F32, tag="bc", bufs=2)
                    for hb in range(2):
                        dn = ps.tile([128, 512], F32, name="dn",
                                     tag="aux", bufs=2)
                        nc.tensor.matmul(
                            dn[0:1, :],
                            ones_v,
                            a[:, 512 * hb:512 * (hb + 1)],
                            start=True,
                            stop=True,
                            tile_position=(0, 0),
                        )
                        nc.vector.reciprocal_approx_fast(
                            out=rec[0:1, 512 * hb:512 * (hb + 1)],
                            in_=dn[0:1, :],
                        )
                    nc.gpsimd.partition_broadcast(
                        bc, rec[0:1, :], channels=128
                    )
                    bc_t[(w, u)] = bc

            def emit_yu(gs):
                """col-tiled concurrent pair: yu_A(gs) || yu_B(gs)."""
                if gs < 0 or gs >= NG:
                    return
                p, tb, i, u = win(gs)
                w = gs // 32
                need(("v", i))
                e = exp_t.pop(gs)
                for hb in range(2):   # A then B adjacent -> concurrent
                    pb = 64 * hb
                    nc.tensor.matmul(
                        yu[pb:pb + 64, u * 512:(u + 1) * 512],
                        v_sb[:, i, 128 * p + pb:128 * p + pb + 64],
                        e[:, 512 * hb:512 * (hb + 1)],
                        start=(i == 0),
                        stop=(i == TT - 1),
                        skip_group_check=True,
                    )
                if i == TT - 1:
                    # u-half of window w complete: normalize both heads
                    bc = bc_t.pop((w, u))
                    with nc.allow_low_precision(reason="f16 y"):
                        for hb in range(2):
                            pb = 64 * hb
                            nc.vector.tensor_mul(
                                out=yt[pb:pb + 64, p,
                                       tb * 1024 + u * 512:
                                       tb * 1024 + (u + 1) * 512],
                                in0=yu[pb:pb + 64,
                                       u * 512:(u + 1) * 512],
                                in1=bc[pb:pb + 64,
                                       512 * hb:512 * (hb + 1)],
                            )

            # ---------- startup ----------
            # wq first on the sync queue (prompt), transposes next, wv
            # sequenced after them so its 2MB transfer doesn't contend
            # with the transpose stream.
            nc.sync.dma_start(out=wq_sb, in_=wq_ap)
            for tb2 in range(2):
                for kc in range(KC):
                    nc.sync.dma_start_transpose(
                        out=xt[kc][:, 1024 * tb2:1024 * (tb2 + 1)],
                        in_=x_ap[1024 * tb2:1024 * (tb2 + 1),
                                 kc * 128:(kc + 1) * 128],
                    )
            nc.sync.dma_start(out=wv_sb, in_=wv_ap)
            force_chain(g_qk(4, 0))   # K^T pair 0, s 0:512
            force_chain(g_qk(0, 0))   # Q^T pair 0, t 0:512
            v_gens = {i: g_v(i) for i in range(TT)}

            startup_forced = {
                48: [lambda: nc.gpsimd.dma_start(out=wo_sb, in_=wo_ap)],
            }
            # remaining pair-0 chains: need() in emit_sc backstops
            push_fill(("qk", 4, 1), g_qk(4, 1), deadline=1)
            push_fill(("qk", 4, 2), g_qk(4, 2), deadline=5)
            push_fill(("qk", 4, 3), g_qk(4, 3), deadline=9)
            push_fill(("qk", 0, 1), g_qk(0, 1), deadline=13)

            def fill_pushes(gs):
                """push new fill chains at window starts (gs units)."""
                w = gs // 32
                if gs % 32 == 0:
                    p, tb = w // 2, w % 2
                    if p < 3:
                        jt = (p + 1) if tb == 0 else (4 + p + 1)
                        base = 32 * (2 * (p + 1))
                        for tbc in range(4):
                            if jt < 4:
                                # Q(tbc): window tb=tbc//2, u-half tbc%2
                                dl = base + 32 * (tbc // 2) \
                                    + 16 * (tbc % 2) - 7
                            else:
                                # K(tbc): first consumed at i = 4*tbc
                                dl = base + 4 * tbc - 7
                            push_fill(("qk", jt, tbc), g_qk(jt, tbc),
                                      deadline=dl)
                    if w == 0:
                        push_fill(("qk", 0, 2), g_qk(0, 2), deadline=29)
                        push_fill(("qk", 0, 3), g_qk(0, 3), deadline=45)
                # o(tb0): yt pair-3 tb0 normalized at yu of gs=32*6+31+LAG
                if gs == 32 * 6 + 31 + LAG + 1:
                    for tt in range(8):
                        for u in range(2):
                            push_fill(("o", tt, u), g_o(tt, u),
                                      deadline=100000)

            # v chains keyed for need(); deadline = consuming yu slot
            for i_ in sorted(v_gens):
                push_fill(("v", i_), v_gens.pop(i_),
                          deadline=i_ + LAG - 2)

            # sc(0) must exist before exp(0)
            emit_sc(0)

            # ---------- main loop ----------
            for gs in range(NG + LAG + 1):
                first_win = gs < 32
                if gs < NG:
                    fill_pushes(gs)
                emit_exp(gs)
                emit_acc(gs)
                emit_sc(gs + 1)
                emit_yu(gs - LAG)
                for fn in startup_forced.get(gs, ()):
                    fn()
                advance_fills(300 if first_win else 420)

            # ---------- tail: output projection for tb=1 ----------
            while fills:
                advance_fills(10000)
            for tt in range(8, 16):
                for u in range(2):
                    force_chain(g_o(tt, u))
            if _DEBUG:
                for jt in range(8):
                    nc.sync.dma_start(out=qkt_d.ap()[:, jt, :],
                                      in_=qkt[jt])
                nc.sync.dma_start(out=v_d.ap(), in_=v_sb)
                nc.sync.dma_start(out=yt_d.ap(), in_=yt)

    nc.compile()
    return nc


def make_in_maps(x, w_qkv, w_o):
    in_maps = []
    for c in range(8):
        b, gg = c // 2, c % 2
        in_maps.append({
            "x": np.ascontiguousarray(x[b], dtype=np.float16),
            "wq": np.ascontiguousarray(
                w_qkv[:, 512 * gg:512 * (gg + 1)], dtype=np.float16),
            "wk": np.ascontiguousarray(
                w_qkv[:, 1024 + 512 * gg:1024 + 512 * (gg + 1)],
                dtype=np.float16),
            "wv": np.ascontiguousarray(
                w_qkv[:, 2048 + 512 * gg:2048 + 512 * (gg + 1)],
                dtype=np.float16),
            "wo": np.ascontiguousarray(
                w_o[512 * gg:512 * (gg + 1), :], dtype=np.float16),
        })
    return in_maps


def kernel(x, w_qkv, w_o, _trace=False, _trace_kwargs=None):
    x = np.asarray(x)
    w_qkv = np.asarray(w_qkv)
    w_o = np.asarray(w_o)
    if "nc" not in _CACHE:
        _CACHE["nc"] = build_nc()
    nc = _CACHE["nc"]
    in_maps = make_in_maps(x, w_qkv, w_o)
    res = run_bass_kernel_spmd(
        nc, in_maps, core_ids=list(range(8)),
        trace=_trace, **(_trace_kwargs or {}),
    )
    out = np.empty((4, T, D), np.float32)
    for b in range(4):
        out[b] = (res.results[2 * b]["out"].astype(np.float32)
                  + res.results[2 * b + 1]["out"].astype(np.float32))
    if _trace:
        _CACHE["last_res"] = res
    return out


# revision 56
# speedup vs baseline: 1.0104x; 1.0104x over previous
"""MHA kernel for Trainium2, 8 NeuronCores — dense ping-pong pipeline v4.

Problem: B=4, T=2048, D=1024, H=16, HD=64 fp32 multi-head attention
  qkv = x @ w_qkv ; attention per head ; out = y @ w_o

Sharding: core c handles batch b = c//2 and head-group g = c%2 (8 of the 16
heads). Each core computes its 8 heads' attention output projected through
the matching w_o row-slice, producing a partial [T, D] f16 output; the host
sums the two partials per batch (row-parallel output projection).

v4 structure: 256 micro-slots gs = (window w = gs//32, s-tile i = (gs//2)%16,
t-half u = gs%2). Two PSUM score tiles [128, 1024] ping-pong by gs parity,
so scores(gs+1) overlap exp(gs) with NO write-after-read stall — the ACT
engine streams one [128, 1024] exp per slot back-to-back (1146ns), and the
PE stays ~100% busy (which also keeps the HAM clock-gate at 2.4 GHz: the
v3 variant measured 42% of the kernel at K=4/8 half-clock because per-slot
idle gaps re-throttled the array).

Per slot gs, emission (= program) order:
  ACT: exp(gs)            reads sc[gs%2] (A-head cols 0:512 | B cols 512:)
  DVE: acc_u(gs) += e(gs) f16 exp-sum per t-half
  PE : yu pair (gs-LAG)   col-tiled A||B concurrent into yu [128, 1024]
  PE : fills              QKV / O projection chains, deadline-ordered
  PE : scores(gs+1)       one row-tiled A||B concurrent pair into sc[1-gs%2]
At i==15 per u: denominator ones-matmuls ([1,512] chunks via aux psum) +
reciprocal into rec[1,2048]; at u==1 one gpsimd partition_broadcast to
bc[128,2048]; the [64,1024] normalize multiply per head lands in yt when
that head's yu finishes (LAG slots later).
"""
import sys

if "/opt/trn_rl_repo" not in sys.path:
    sys.path.insert(0, "/opt/trn_rl_repo")

import heapq

import numpy as np

import concourse.bass as bass
import concourse.mybir as mybir
import concourse.tile as tile
from concourse import bacc
from concourse.bass_isa import ReduceOp
from concourse.bass_utils import run_bass_kernel_spmd

T = 2048
D = 1024
NH = 8          # heads per core
HD = 64
KC = D // 128   # 8 contraction chunks
TT = T // 128   # 16 s tiles
NP = NH // 2    # 4 head pairs
NW = 2 * NP     # 8 windows: w = 2*p + tb
NG = NW * TT * 2  # 256 micro-slots
LAG = 6         # yu lags exp by LAG micro-slots
F32 = mybir.dt.float32
F16 = mybir.dt.float16

_CACHE = {}
_DEBUG = False


def build_nc():
    nc = bacc.Bacc(
        "TRN2",
        target_bir_lowering=False,
        debug=False,
        enable_asserts=False,
        num_devices=8,
    )
    x_d = nc.dram_tensor("x", [T, D], F16, kind="ExternalInput")
    wq_d = nc.dram_tensor("wq", [D, 512], F16, kind="ExternalInput")
    wk_d = nc.dram_tensor("wk", [D, 512], F16, kind="ExternalInput")
    wv_d = nc.dram_tensor("wv", [D, 512], F16, kind="ExternalInput")
    wo_d = nc.dram_tensor("wo", [512, D], F16, kind="ExternalInput")
    out_d = nc.dram_tensor("out", [T, D], F16, kind="ExternalOutput")
    if _DEBUG:
        qkt_d = nc.dram_tensor("qkt_dump", [128, 8, T], F16,
                               kind="ExternalOutput")
        v_d = nc.dram_tensor("v_dump", [128, TT, 512], F16,
                             kind="ExternalOutput")
        yt_d = nc.dram_tensor("yt_dump", [128, NP, T], F16,
                              kind="ExternalOutput")

    x_ap = x_d.ap()
    wq_ap = wq_d.ap().rearrange("(kc p) j -> p kc j", p=128)   # [128, 8, 512]
    wk_ap = wk_d.ap().rearrange("(kc p) j -> p kc j", p=128)
    wv_ap = wv_d.ap().rearrange("(kc p) j -> p kc j", p=128)
    wo_ap = wo_d.ap().rearrange("(c p) n -> p c n", p=128)     # [128, 4, 1024]

    def win(gs):
        """micro-slot -> (pair, tb, i, u); u-outer within a window."""
        w, r = gs // 32, gs % 32
        return w // 2, w % 2, r % 16, r // 16

    with tile.TileContext(nc) as tc:
        with (
            tc.sbuf_pool(name="sb", bufs=1) as sb,
            tc.psum_pool(name="ps", bufs=1) as ps,
        ):
            # ---- persistent sbuf (separate tiles per logical slice) ----
            xt = [sb.tile([128, T], F16, name=f"xt{kc}") for kc in range(KC)]
            qkt = [sb.tile([128, T], F16, name=f"qkt{jt}") for jt in range(8)]
            v_sb = sb.tile([128, TT, 512], F16)      # V [s-part, s-chunk, j]
            yt = sb.tile([128, NP, T], F16)          # y^T [dy, pair, t]
            wq_sb = sb.tile([128, KC, 512], F16)
            wk_sb = sb.tile([128, KC, 512], F16)
            wv_sb = sb.tile([128, KC, 512], F16)
            wo_sb = sb.tile([128, 4, D], F16)
            ones_v = sb.tile([128, 1], F16)
            nc.vector.memset(ones_v, 1.0)
            warm = sb.tile([1, 32], F16)
            nc.vector.memset(warm, 0.0)
            nc.scalar.activation(
                warm, warm, mybir.ActivationFunctionType.Exp, scale=0.125
            )

            nc.scalar.dma_start(out=wk_sb, in_=wk_ap)
            nc.scalar.dma_start(out=wv_sb, in_=wv_ap)

            # ---------- fill chains (QKV / O projections) ----------
            fills = []     # heap of (deadline_slot, seq, key)
            fseq = [0]
            pending = {}

            def g_qk(jt, tbc):
                """qkt[jt][:, tbc*512:(tbc+1)*512] = (w chunk)^T @ xt."""
                aux = ps.tile([128, 512], F32, name="qkps", tag="aux", bufs=2)
                w_sb = wq_sb if jt < 4 else wk_sb
                j4 = jt % 4
                for kc in range(KC):
                    nc.tensor.matmul(
                        aux,
                        w_sb[:, kc, j4 * 128:(j4 + 1) * 128],
                        xt[kc][:, tbc * 512:(tbc + 1) * 512],
                        start=(kc == 0),
                        stop=(kc == KC - 1),
                        skip_group_check=True,
                    )
                    yield 230
                nc.vector.tensor_copy(
                    out=qkt[jt][:, tbc * 512:(tbc + 1) * 512], in_=aux
                )

            def g_v(i):
                aux = ps.tile([128, 512], F32, name="vps", tag="aux", bufs=2)
                for kc in range(KC):
                    nc.tensor.matmul(
                        aux,
                        xt[kc][:, i * 128:(i + 1) * 128],
                        wv_sb[:, kc, :],
                        start=(kc == 0),
                        stop=(kc == KC - 1),
                        skip_group_check=True,
                    )
                    yield 230
                nc.vector.tensor_copy(out=v_sb[:, i, :], in_=aux)

            def g_o(tt, u):
                aux = ps.tile([128, 512], F32, name="ops", tag="aux", bufs=2)
                for c4 in range(4):
                    nc.tensor.matmul(
                        aux,
                        yt[:, c4, tt * 128:(tt + 1) * 128],
                        wo_sb[:, c4, u * 512:(u + 1) * 512],
                        start=(c4 == 0),
                        stop=(c4 == 3),
                        skip_group_check=True,
                    )
                    yield 230
                o_sb = sb.tile([128, 512], F16, tag="osb", bufs=2)
                with nc.allow_low_precision(reason="f16 partial output"):
                    nc.vector.tensor_copy(out=o_sb, in_=aux)
                nc.sync.dma_start(
                    out=out_d.ap()[
                        tt * 128:(tt + 1) * 128, u * 512:(u + 1) * 512
                    ],
                    in_=o_sb,
                )

            def push_fill(key, gen, deadline=100000):
                pending[key] = gen
                heapq.heappush(fills, (deadline, fseq[0], key))
                fseq[0] += 1

            def advance_fills(budget):
                while fills and budget > 0:
                    key = fills[0][2]
                    gen = pending.get(key)
                    if gen is None:
                        heapq.heappop(fills)
                        continue
                    try:
                        budget -= next(gen)
                    except StopIteration:
                        del pending[key]
                        heapq.heappop(fills)

            def need(key):
                gen = pending.pop(key, None)
                if gen is not None:
                    for _ in gen:
                        pass

            def force_chain(gen):
                for _ in gen:
                    pass

            # ---------- attention state ----------
            # two ping-pong score tiles; A-head cols 0:512, B cols 512:1024
            scb = [ps.tile([128, 1024], F32, name=f"sc{b}", tag=f"sc{b}",
                           bufs=1) for b in range(2)]
            yu = ps.tile([128, 1024], F32, name="yu", tag="yu", bufs=1)
            exp_t = {}       # gs -> sbuf exp tile [128, 1024] (A|B)
            acc_t = [None, None]   # per-u running f16 exp-sum [128, 1024]
            bc_t = {}        # (w, u) -> 1/denominator [128, 1024] (A|B)

            def emit_sc(gs):
                """scores for micro-slot gs: one row-tiled A||B pair."""
                if gs < 0 or gs >= NG:
                    return
                p, tb, i, u = win(gs)
                need(("qk", 4 + p, i // 4))
                need(("qk", p, 2 * tb + u))
                buf = scb[gs % 2]
                tcol = tb * 1024 + u * 512
                for hb in range(2):   # A then B adjacent -> concurrent
                    pb = 64 * hb
                    nc.tensor.matmul(
                        buf[:, 512 * hb:512 * (hb + 1)],
                        qkt[4 + p][pb:pb + 64, i * 128:(i + 1) * 128],
                        qkt[p][pb:pb + 64, tcol:tcol + 512],
                        start=True,
                        stop=True,
                    )

            def emit_exp(gs):
                if gs < 0 or gs >= NG:
                    return
                e = sb.tile([128, 1024], F16, tag="exp", bufs=8)
                nc.scalar.activation(
                    e, scb[gs % 2], mybir.ActivationFunctionType.Exp,
                    scale=0.125,
                )
                exp_t[gs] = e

            def emit_acc(gs):
                if gs < 0 or gs >= NG:
                    return
                p, tb, i, u = win(gs)
                a = sb.tile([128, 1024], F16, tag=f"accu{u}", bufs=2)
                if i == 0:
                    nc.vector.tensor_copy(out=a, in_=exp_t[gs])
                else:
                    with nc.allow_low_precision(reason="f16 exp-sum"):
                        nc.vector.tensor_add(out=a, in0=acc_t[u],
                                             in1=exp_t[gs])
                acc_t[u] = a
                if i == TT - 1:
                    # denominator chunks for this u: A and B halves via
                    # ones-matmul -> reciprocal -> gpsimd broadcast.
                    w = gs // 32
                    rec = sb.tile([1, 1024], F32, tag="rec", bufs=2)
                    bc = sb.tile([128, 1024], F32, tag="bc", bufs=2)
                    for hb in range(2):
                        dn = ps.tile([128, 512], F32, name="dn",
                                     tag="aux", bufs=2)
                        nc.tensor.matmul(
                            dn[0:1, :],
                            ones_v,
                            a[:, 512 * hb:512 * (hb + 1)],
                            start=True,
                            stop=True,
                            tile_position=(0, 0),
                        )
                        nc.vector.reciprocal_approx_fast(
                            out=rec[0:1, 512 * hb:512 * (hb + 1)],
                            in_=dn[0:1, :],
                        )
                    nc.gpsimd.partition_broadcast(
                        bc, rec[0:1, :], channels=128
                    )
                    bc_t[(w, u)] = bc

            def emit_yu(gs):
                """col-tiled concurrent pair: yu_A(gs) || yu_B(gs)."""
                if gs < 0 or gs >= NG:
                    return
                p, tb, i, u = win(gs)
                w = gs // 32
                need(("v", i))
                e = exp_t.pop(gs)
                for hb in range(2):   # A then B adjacent -> concurrent
                    pb = 64 * hb
                    nc.tensor.matmul(
                        yu[pb:pb + 64, u * 512:(u + 1) * 512],
                        v_sb[:, i, 128 * p + pb:128 * p + pb + 64],
                        e[:, 512 * hb:512 * (hb + 1)],
                        start=(i == 0),
                        stop=(i == TT - 1),
                        skip_group_check=True,
                    )
                if i == TT - 1:
                    # u-half of window w complete: normalize both heads
                    bc = bc_t.pop((w, u))
                    with nc.allow_low_precision(reason="f16 y"):
                        for hb in range(2):
                            pb = 64 * hb
                            nc.vector.tensor_mul(
                                out=yt[pb:pb + 64, p,
                                       tb * 1024 + u * 512:
                                       tb * 1024 + (u + 1) * 512],
                                in0=yu[pb:pb + 64,
                                       u * 512:(u + 1) * 512],
                                in1=bc[pb:pb + 64,
                                       512 * hb:512 * (hb + 1)],
                            )

            # ---------- startup ----------
            # wq first on the sync queue (prompt), transposes next, wv
            # sequenced after them so its 2MB transfer doesn't contend
            # with the transpose stream.
            nc.sync.dma_start(out=wq_sb, in_=wq_ap)
            for tb2 in range(2):
                for kc in range(KC):
                    nc.sync.dma_start_transpose(
                        out=xt[kc][:, 1024 * tb2:1024 * (tb2 + 1)],
                        in_=x_ap[1024 * tb2:1024 * (tb2 + 1),
                                 kc * 128:(kc + 1) * 128],
                    )
            force_chain(g_qk(4, 0))   # K^T pair 0, s 0:512
            force_chain(g_qk(0, 0))   # Q^T pair 0, t 0:512
            v_gens = {i: g_v(i) for i in range(TT)}

            startup_forced = {
                48: [lambda: nc.gpsimd.dma_start(out=wo_sb, in_=wo_ap)],
            }
            # remaining pair-0 chains: need() in emit_sc backstops
            push_fill(("qk", 4, 1), g_qk(4, 1), deadline=1)
            push_fill(("qk", 4, 2), g_qk(4, 2), deadline=5)
            push_fill(("qk", 4, 3), g_qk(4, 3), deadline=9)
            push_fill(("qk", 0, 1), g_qk(0, 1), deadline=13)

            def fill_pushes(gs):
                """push new fill chains at window starts (gs units)."""
                w = gs // 32
                if gs % 32 == 0:
                    p, tb = w // 2, w % 2
                    if p < 3:
                        jt = (p + 1) if tb == 0 else (4 + p + 1)
                        base = 32 * (2 * (p + 1))
                        for tbc in range(4):
                            if jt < 4:
                                # Q(tbc): window tb=tbc//2, u-half tbc%2
                                dl = base + 32 * (tbc // 2) \
                                    + 16 * (tbc % 2) - 7
                            else:
                                # K(tbc): first consumed at i = 4*tbc
                                dl = base + 4 * tbc - 7
                            push_fill(("qk", jt, tbc), g_qk(jt, tbc),
                                      deadline=dl)
                    if w == 0:
                        push_fill(("qk", 0, 2), g_qk(0, 2), deadline=29)
                        push_fill(("qk", 0, 3), g_qk(0, 3), deadline=45)
                # o(tb0): yt pair-3 tb0 normalized at yu of gs=32*6+31+LAG
                if gs == 32 * 6 + 31 + LAG + 1:
                    for tt in range(8):
                        for u in range(2):
                            push_fill(("o", tt, u), g_o(tt, u),
                                      deadline=100000)

            # v chains keyed for need(); deadline = consuming yu slot
            for i_ in sorted(v_gens):
                push_fill(("v", i_), v_gens.pop(i_),
                          deadline=i_ + LAG - 2)

            # sc(0) must exist before exp(0)
            emit_sc(0)

            # ---------- main loop ----------
            for gs in range(NG + LAG + 1):
                first_win = gs < 32
                if gs < NG:
                    fill_pushes(gs)
                emit_exp(gs)
                emit_acc(gs)
                emit_sc(gs + 1)
                emit_yu(gs - LAG)
                for fn in startup_forced.get(gs, ()):
                    fn()
                advance_fills(300 if first_win else 420)

            # ---------- tail: output projection for tb=1 ----------
            while fills:
                advance_fills(10000)
            for tt in range(8, 16):
                for u in range(2):
                    force_chain(g_o(tt, u))
            if _DEBUG:
                for jt in range(8):
                    nc.sync.dma_start(out=qkt_d.ap()[:, jt, :],
                                      in_=qkt[jt])
                nc.sync.dma_start(out=v_d.ap(), in_=v_sb)
                nc.sync.dma_start(out=yt_d.ap(), in_=yt)

    nc.compile()
    return nc


def make_in_maps(x, w_qkv, w_o):
    in_maps = []
    for c in range(8):
        b, gg = c // 2, c % 2
        in_maps.append({
            "x": np.ascontiguousarray(x[b], dtype=np.float16),
            "wq": np.ascontiguousarray(
                w_qkv[:, 512 * gg:512 * (gg + 1)], dtype=np.float16),
            "wk": np.ascontiguousarray(
                w_qkv[:, 1024 + 512 * gg:1024 + 512 * (gg + 1)],
                dtype=np.float16),
            "wv": np.ascontiguousarray(
                w_qkv[:, 2048 + 512 * gg:2048 + 512 * (gg + 1)],
                dtype=np.float16),
            "wo": np.ascontiguousarray(
                w_o[512 * gg:512 * (gg + 1), :], dtype=np.float16),
        })
    return in_maps


def kernel(x, w_qkv, w_o, _trace=False, _trace_kwargs=None):
    x = np.asarray(x)
    w_qkv = np.asarray(w_qkv)
    w_o = np.asarray(w_o)
    if "nc" not in _CACHE:
        _CACHE["nc"] = build_nc()
    nc = _CACHE["nc"]
    in_maps = make_in_maps(x, w_qkv, w_o)
    res = run_bass_kernel_spmd(
        nc, in_maps, core_ids=list(range(8)),
        trace=_trace, **(_trace_kwargs or {}),
    )
    out = np.empty((4, T, D), np.float32)
    for b in range(4):
        out[b] = (res.results[2 * b]["out"].astype(np.float32)
                  + res.results[2 * b + 1]["out"].astype(np.float32))
    if _trace:
        _CACHE["last_res"] = res
    return out
